# revision 6
# baseline (speedup 1.0000x reference)
"""DoubleAttention forward on 8 Trainium2 NeuronCores.

Reference (per sample, x: [512, 4096] after flattening h*w):
    A = wA @ x + bA            [128, n]
    B = wB @ x + bB            [128, n]
    V = wV @ x + bV            [128, n]
    M = softmax(B, axis=ch)    [128, n]
    W = softmax(V, axis=ch)    [128, n]
    gd = A @ M.T               [128, 128]
    Z = gd @ W                 [128, n]
    out = wR @ Z + bR          [512, n]

Sharding: data-parallel over batch, 16 samples -> 8 cores x 2 each.

Implementation notes:
  - All matmul inputs fp16 (exact products, fp32 PSUM accumulation);
    x / weights converted host-side, which also halves the input DMA.
  - Transposed layout: per 128-wide n-tile, P1[n, A|B|V] = x_chunk.T @
    [wA.T|wB.T|wV.T], so the channel softmax is a free-dim op.
  - n-tiles processed in PAIRS sharing one 2-bank PSUM tile so the
    elementwise ops run at 2x width (amortizes fixed per-op cost).
  - Softmax normalization is NOT materialized for B: the ACT exp writes
    exp(logit - 12*ln2) fp16 straight into the M/W store, and 1/sum is
    folded into the A-tile evacuation (a per-partition scale in this
    layout, since gd[m,k] = sum_n (A[m,n]/s[n]) expB[k,n]).  Only the
    V half gets an in-place normalize (on the otherwise idle GPSIMD).
  - gd^T accumulates on PE across tiles (2-pair emission lag keeps PE
    dense while the newest pair's softmax drains).
  - Phase 3 transposes W^T tiles back via PE, then Z = gdT.T @ W and
    out = wRT.T @ Z (fp16, N=512 moving dim).
  - Output staged fp16 and upcast host-side (halves the store DMA).
  - Biases fold in as rank-1 PSUM-accumulate matmuls / ACT bias adds,
    all skipped when the bias vectors are zero (the common case).
"""

import sys

if "/opt/trn_rl_repo" not in sys.path:
    sys.path.insert(0, "/opt/trn_rl_repo")

import numpy as np

import concourse.bacc as bacc
import concourse.tile as tile
from concourse import masks, mybir
from concourse.bass_utils import run_bass_kernel_spmd

N_CORES = 8
B_GLOBAL = 16
B_LOC = B_GLOBAL // N_CORES
C_IN, C_M, C_N = 512, 128, 128
H = W = 64
N = H * W                      # 4096 spatial positions
NT = N // 128                  # 32 tiles of 128 positions
NP = NT // 2                   # 16 tile-pairs
KC = C_IN // 128               # 4 contraction chunks
NE = N // 512                  # 8 x-load slices of 512 columns
NG = N // 512                  # 8 output groups of 512 positions
SHIFT = float(-12.0 * np.log(2.0))   # exp downshift so fp16 never overflows
F32 = mybir.dt.float32
F16 = mybir.dt.float16
EXP = mybir.ActivationFunctionType.Exp
IDENT = mybir.ActivationFunctionType.Identity


def _build(has_bias_abv: bool, has_bias_r: bool):
    nc = bacc.Bacc("TRN2", target_bir_lowering=False, debug=False)

    x_d = nc.dram_tensor("x", (B_LOC, C_IN, N), F16, kind="ExternalInput")
    wcat_d = nc.dram_tensor("wcat", (KC, 128, 384), F16, kind="ExternalInput")
    wrt_d = nc.dram_tensor("wrt", (128, C_IN), F16, kind="ExternalInput")
    if has_bias_abv:
        bcat_d = nc.dram_tensor("bcat", (1, 384), F16, kind="ExternalInput")
    if has_bias_r:
        brt_d = nc.dram_tensor("brt", (128, KC), F32, kind="ExternalInput")
    out_d = nc.dram_tensor("out", (B_LOC, C_IN, N), F16, kind="ExternalOutput")

    with tile.TileContext(nc) as tc:
        with (
            tc.tile_pool(name="const", bufs=1) as constp,
            tc.tile_pool(name="xq", bufs=2 * NE) as xqp,
            tc.tile_pool(name="mw", bufs=2 * NP) as mwp,
            tc.tile_pool(name="at", bufs=5) as atp,
            tc.tile_pool(name="st", bufs=4) as stp,
            tc.tile_pool(name="gds", bufs=2) as gdsp,
            tc.tile_pool(name="wsb", bufs=3) as wsbp,
            tc.tile_pool(name="zsb", bufs=2) as zsbp,
            tc.tile_pool(name="osb", bufs=2) as osbp,
            tc.tile_pool(name="pP", bufs=2, space="PSUM") as pP,
            tc.tile_pool(name="pG", bufs=1, space="PSUM") as pG,
            tc.tile_pool(name="pW", bufs=1, space="PSUM") as pW,
            tc.tile_pool(name="pZO", bufs=2, space="PSUM") as pZO,
        ):
            wcat = constp.tile([128, KC, 384], F16)
            nc.sync.dma_start(wcat[:], wcat_d.ap().rearrange("k p j -> p k j"))
            wrt = constp.tile([128, C_IN], F16)
            nc.sync.dma_start(wrt[:], wrt_d[:])
            ident16 = constp.tile([128, 128], F16)
            masks.make_identity(nc, ident16[:])
            shift = constp.tile([128, 1], F32)
            nc.gpsimd.memset(shift[:], SHIFT)
            if has_bias_abv:
                bcat = constp.tile([1, 384], F16)
                nc.sync.dma_start(bcat[:], bcat_d[:])
                ones1 = constp.tile([1, 128], F16)
                nc.gpsimd.memset(ones1[:], 1.0)
            if has_bias_r:
                brt = constp.tile([128, KC], F32)
                nc.sync.dma_start(brt[:], brt_d[:])

            for s in range(B_LOC):
                # ---- phase 1: projections + channel softmax + gdT ----
                xq = [
                    xqp.tile([128, KC, 512], F16, tag="xq", name=f"xq{s}_{e}")
                    for e in range(NE)
                ]
                src = x_d[s].rearrange("(k p) n -> p k n", p=128)
                for e in range(NE):
                    nc.sync.dma_start(xq[e][:], src[:, :, e * 512:(e + 1) * 512])

                mwts = [
                    mwp.tile([128, 2, 2, 128], F16, tag="mw", name=f"mw{s}_{i}")
                    for i in range(NP)
                ]
                ats = {}
                gdt = pG.tile([128, 128], F32)

                def emit_gd(i, first, last):
                    for j in (0, 1):
                        nc.tensor.matmul(
                            gdt[:], mwts[i][:, j, 0, :], ats[i][:, j, :],
                            start=(first and j == 0), stop=(last and j == 1),
                            skip_group_check=True,
                        )

                for i in range(NP):
                    p1 = pP.tile([128, 1024], F32)
                    p1v = p1.rearrange("p (j r c) -> p j r c", j=2, c=128)
                    for j in (0, 1):
                        nt = 2 * i + j
                        e, col = nt // 4, (nt % 4) * 128
                        dst = p1[:, j * 512:j * 512 + 384]
                        for k in range(KC):
                            nc.tensor.matmul(
                                dst, xq[e][:, k, col:col + 128], wcat[:, k, :],
                                start=(k == 0),
                                stop=(k == KC - 1 and not has_bias_abv),
                            )
                        if has_bias_abv:
                            nc.tensor.matmul(
                                dst, ones1[:], bcat[:], start=False, stop=True
                            )

                    # gd for pair i-2: keeps PE busy while softmax(i-1..i) runs
                    if i >= 2:
                        emit_gd(i - 2, first=(i == 2), last=False)

                    mw = mwts[i]
                    # exp(logit + SHIFT) -> fp16, straight into the M/W store
                    nc.scalar.activation(mw[:], p1v[:, :, 1:3, :], EXP,
                                         bias=shift[:])
                    sums = stp.tile([128, 2, 2], F32, tag="sums")
                    nc.vector.reduce_sum(sums[:], mw[:],
                                         axis=mybir.AxisListType.X)
                    rec = stp.tile([128, 2, 2], F32, tag="rec")
                    nc.vector.reciprocal(rec[:], sums[:])
                    # A scaled by 1/sum(expB) -- the entire B normalization
                    at = atp.tile([128, 2, 128], F16)
                    nc.vector.tensor_mul(
                        at[:], p1v[:, :, 0, :],
                        rec[:, :, 0:1].broadcast_to([128, 2, 128]),
                    )
                    ats[i] = at
                    # V half normalized in place (on the idle GPSIMD)
                    nc.gpsimd.tensor_mul(
                        mw[:, :, 1, :], mw[:, :, 1, :],
                        rec[:, :, 1:2].broadcast_to([128, 2, 128]),
                    )

                emit_gd(NP - 2, first=(NP == 2), last=False)
                emit_gd(NP - 1, first=False, last=True)
                gdts = gdsp.tile([128, 128], F16)
                nc.vector.tensor_copy(gdts[:], gdt[:])

                # ---- phase 3: transpose W, distribute, reconstruct ----
                wsbs = []
                osb = None

                def emit_group(g, osb_t):
                    zun = pZO.tile([128, 512], F32, tag="pzo", name=f"zun{s}_{g}")
                    nc.tensor.matmul(
                        zun[:], gdts[:], wsbs[g][:], start=True, stop=True
                    )
                    zsb = zsbp.tile([128, 512], F16, name=f"zsb{s}_{g}")
                    nc.scalar.copy(zsb[:], zun[:])
                    half = (g % 2) * 512
                    for k in range(KC):
                        ock = pZO.tile(
                            [128, 512], F32, tag="pzo", name=f"ock{s}_{g}_{k}"
                        )
                        nc.tensor.matmul(
                            ock[:], wrt[:, k * 128:(k + 1) * 128], zsb[:],
                            start=True, stop=True,
                        )
                        dst = osb_t[:, k, half:half + 512]
                        if has_bias_r:
                            nc.scalar.activation(
                                dst, ock[:], IDENT, bias=brt[:, k:k + 1]
                            )
                        elif k == 3 or (k == 2 and g % 2 == 1):
                            nc.vector.tensor_copy(dst, ock[:])
                        else:
                            nc.scalar.copy(dst, ock[:])
                    if g % 2 == 1:
                        dsto = out_d[s].rearrange("(k p) n -> p k n", p=128)
                        c0 = (g - 1) * 512
                        nc.sync.dma_start(dsto[:, :, c0:c0 + 1024], osb_t[:])

                for g in range(NG):
                    wpt = pW.tile([128, 512], F16)
                    for j in range(KC):
                        nt = 4 * g + j
                        nc.tensor.transpose(
                            wpt[:, j * 128:(j + 1) * 128],
                            mwts[nt // 2][:, nt % 2, 1, :],
                            ident16[:],
                        )
                    wsb = wsbp.tile([128, 512], F16, tag="wsb", name=f"wsb{s}_{g}")
                    nc.vector.tensor_copy(wsb[:], wpt[:])
                    wsbs.append(wsb)

                    if g % 2 == 0:
                        osb = osbp.tile(
                            [128, KC, 1024], F16, tag="osb", name=f"osb{s}_{g}"
                        )
                    if g > 0:
                        emit_group(g - 1, osb if g % 2 == 1 else prev_osb)
                    prev_osb = osb
                emit_group(NG - 1, osb)

    nc.compile()
    return nc


_CACHE = {}


def _get_nc(has_bias_abv: bool, has_bias_r: bool):
    key = (has_bias_abv, has_bias_r)
    if key not in _CACHE:
        _CACHE[key] = _build(*key)
    return _CACHE[key]


def _run(inputs, trace=False, **spmd_kwargs):
    x = np.asarray(inputs["x"])
    b, c, h, w = x.shape
    assert (b, c, h, w) == (B_GLOBAL, C_IN, H, W), x.shape
    wA = np.asarray(inputs["wA"], np.float32)
    wB = np.asarray(inputs["wB"], np.float32)
    wV = np.asarray(inputs["wV"], np.float32)
    wR = np.asarray(inputs["wR"], np.float32)
    bA = np.asarray(inputs["bA"], np.float32)
    bB = np.asarray(inputs["bB"], np.float32)
    bV = np.asarray(inputs["bV"], np.float32)
    bR = np.asarray(inputs["bR"], np.float32)

    has_bias_abv = bool(np.any(bA) or np.any(bB) or np.any(bV))
    has_bias_r = bool(np.any(bR))
    nc = _get_nc(has_bias_abv, has_bias_r)

    # [KC, 128, 384] : chunk k holds [wA.T | wB.T | wV.T][k*128:(k+1)*128, :]
    wcat = np.concatenate([wA.T, wB.T, wV.T], axis=1).reshape(KC, 128, 3 * 128)
    base = {
        "wcat": np.ascontiguousarray(wcat, dtype=np.float16),
        "wrt": np.ascontiguousarray(wR.T, dtype=np.float16),
    }
    if has_bias_abv:
        base["bcat"] = np.concatenate([bA, bB, bV])[None, :].astype(np.float16)
    if has_bias_r:
        base["brt"] = np.ascontiguousarray(bR.reshape(KC, 128).T, np.float32)

    xf = np.asarray(x, np.float16).reshape(B_GLOBAL, C_IN, N)
    in_maps = [
        dict(base, x=np.ascontiguousarray(xf[ci * B_LOC:(ci + 1) * B_LOC]))
        for ci in range(N_CORES)
    ]
    res = run_bass_kernel_spmd(
        nc, in_maps, core_ids=list(range(N_CORES)), trace=trace, **spmd_kwargs
    )
    out = np.concatenate(
        [res.results[ci]["out"].astype(np.float32) for ci in range(N_CORES)],
        axis=0,
    )
    return out.reshape(B_GLOBAL, C_IN, H, W), res


def kernel(**inputs):
    out, _ = _run(inputs)
    return out


# revision 11
# speedup vs baseline: 1.0942x; 1.0942x over previous
"""DoubleAttention forward on 8 Trainium2 NeuronCores.

Reference (per sample, x: [512, 4096] after flattening h*w):
    A = wA @ x + bA            [128, n]
    B = wB @ x + bB            [128, n]
    V = wV @ x + bV            [128, n]
    M = softmax(B, axis=ch)    [128, n]
    W = softmax(V, axis=ch)    [128, n]
    gd = A @ M.T               [128, 128]
    Z = gd @ W                 [128, n]
    out = wR @ Z + bR          [512, n]

Sharding: data-parallel over batch, 16 samples -> 8 cores x 2 each.

Implementation notes:
  - All matmul inputs fp16 (exact products, fp32 PSUM accumulation);
    x / weights converted host-side, which also halves the input DMA.
  - Transposed layout: per 128-wide n-tile, P1[n, A|B|V] = x_chunk.T @
    [wA.T|wB.T|wV.T], so the channel softmax is a free-dim op.
  - n-tiles processed in PAIRS sharing one 2-bank PSUM tile so the
    elementwise ops run at 2x width (amortizes fixed per-op cost).
  - Softmax normalization is NOT materialized for B: the ACT exp writes
    exp(logit - 12*ln2) fp16 straight into the M/W store, and 1/sum is
    folded into the A-tile evacuation (a per-partition scale in this
    layout, since gd[m,k] = sum_n (A[m,n]/s[n]) expB[k,n]).  Only the
    V half gets an in-place normalize (on the otherwise idle GPSIMD).
  - gd accumulates on PE across tiles (2-pair emission lag keeps PE
    dense while the newest pair's softmax drains).
  - out = wR @ (gd @ W) is reassociated to (wR @ gd) @ W: G^T = gd.T@wR.T
    is one N=512 matmul per sample, so phase 3 is just transpose-W plus
    four direct output matmuls per group (no Z round-trip through PSUM).
  - All x for both samples prefetches up front (SBUF is big enough), so
    the HWDGE queue never parks output-store waits ahead of input loads.
  - Output staged fp16 and upcast host-side (halves the store DMA).
  - Biases fold in as rank-1 PSUM-accumulate matmuls / ACT bias adds,
    all skipped when the bias vectors are zero (the common case).
"""

import sys

if "/opt/trn_rl_repo" not in sys.path:
    sys.path.insert(0, "/opt/trn_rl_repo")

import numpy as np

import concourse.bacc as bacc
import concourse.tile as tile
from concourse import masks, mybir
from concourse.bass_utils import run_bass_kernel_spmd

N_CORES = 8
B_GLOBAL = 16
B_LOC = B_GLOBAL // N_CORES
C_IN, C_M, C_N = 512, 128, 128
H = W = 64
N = H * W                      # 4096 spatial positions
NT = N // 128                  # 32 tiles of 128 positions
NP = NT // 2                   # 16 tile-pairs
KC = C_IN // 128               # 4 contraction chunks
NE = N // 512                  # 8 x-load slices of 512 columns
NG = N // 512                  # 8 output groups of 512 positions
SHIFT = float(-12.0 * np.log(2.0))   # exp downshift so fp16 never overflows
F32 = mybir.dt.float32
F16 = mybir.dt.float16
EXP = mybir.ActivationFunctionType.Exp
IDENT = mybir.ActivationFunctionType.Identity


def _build(has_bias_abv: bool, has_bias_r: bool):
    nc = bacc.Bacc("TRN2", target_bir_lowering=False, debug=False)

    x_d = nc.dram_tensor("x", (B_LOC, C_IN, N), F16, kind="ExternalInput")
    wcat_d = nc.dram_tensor("wcat", (KC, 128, 384), F16, kind="ExternalInput")
    wrt_d = nc.dram_tensor("wrt", (128, C_IN), F16, kind="ExternalInput")
    if has_bias_abv:
        bcat_d = nc.dram_tensor("bcat", (1, 384), F16, kind="ExternalInput")
    if has_bias_r:
        brt_d = nc.dram_tensor("brt", (128, KC), F32, kind="ExternalInput")
    out_d = nc.dram_tensor("out", (B_LOC, C_IN, N), F16, kind="ExternalOutput")

    with tile.TileContext(nc) as tc:
        with (
            tc.tile_pool(name="const", bufs=1) as constp,
            tc.tile_pool(name="xq", bufs=2 * NE) as xqp,
            tc.tile_pool(name="mw", bufs=2 * NP) as mwp,
            tc.tile_pool(name="at", bufs=5) as atp,
            tc.tile_pool(name="st", bufs=4) as stp,
            tc.tile_pool(name="gds", bufs=2) as gdsp,
            tc.tile_pool(name="gts", bufs=2) as gtsp,
            tc.tile_pool(name="wsb", bufs=3) as wsbp,
            tc.tile_pool(name="osb", bufs=2) as osbp,
            tc.tile_pool(name="pP", bufs=2, space="PSUM") as pP,
            tc.tile_pool(name="pG", bufs=1, space="PSUM") as pG,
            tc.tile_pool(name="pW", bufs=1, space="PSUM") as pW,
            tc.tile_pool(name="pO", bufs=2, space="PSUM") as pO,
        ):
            wcat = constp.tile([128, KC, 384], F16)
            nc.sync.dma_start(wcat[:], wcat_d.ap().rearrange("k p j -> p k j"))
            wrt = constp.tile([128, C_IN], F16)
            nc.sync.dma_start(wrt[:], wrt_d[:])
            ident16 = constp.tile([128, 128], F16)
            masks.make_identity(nc, ident16[:])
            shift = constp.tile([128, 1], F32)
            nc.gpsimd.memset(shift[:], SHIFT)
            if has_bias_abv:
                bcat = constp.tile([1, 384], F16)
                nc.sync.dma_start(bcat[:], bcat_d[:])
                ones1 = constp.tile([1, 128], F16)
                nc.gpsimd.memset(ones1[:], 1.0)
            if has_bias_r:
                brt = constp.tile([128, KC], F32)
                nc.sync.dma_start(brt[:], brt_d[:])

            # prefetch ALL of x up front: keeps the HWDGE queue free of
            # store-side waits when the next sample's loads would launch
            xqs = []
            for s in range(B_LOC):
                xq = [
                    xqp.tile([128, KC, 512], F16, tag="xq", name=f"xq{s}_{e}")
                    for e in range(NE)
                ]
                src = x_d[s].rearrange("(k p) n -> p k n", p=128)
                for e in range(NE):
                    nc.sync.dma_start(xq[e][:], src[:, :, e * 512:(e + 1) * 512])
                xqs.append(xq)

            for s in range(B_LOC):
                # ---- phase 1: projections + channel softmax + gd ----
                xq = xqs[s]
                mwts = [
                    mwp.tile([128, 2, 2, 128], F16, tag="mw", name=f"mw{s}_{i}")
                    for i in range(NP)
                ]
                ats = {}
                gdt = pG.tile([128, 128], F32)

                def emit_gd(i, first, last):
                    # gd[m,k] += sum_n (A[m,n]/sB[n]) * expB[k,n]
                    for j in (0, 1):
                        nc.tensor.matmul(
                            gdt[:], ats[i][:, j, :], mwts[i][:, j, 0, :],
                            start=(first and j == 0), stop=(last and j == 1),
                            skip_group_check=True,
                        )

                for i in range(NP):
                    p1 = pP.tile([128, 1024], F32)
                    p1v = p1.rearrange("p (j r c) -> p j r c", j=2, c=128)
                    for j in (0, 1):
                        nt = 2 * i + j
                        e, col = nt // 4, (nt % 4) * 128
                        dst = p1[:, j * 512:j * 512 + 384]
                        for k in range(KC):
                            nc.tensor.matmul(
                                dst, xq[e][:, k, col:col + 128], wcat[:, k, :],
                                start=(k == 0),
                                stop=(k == KC - 1 and not has_bias_abv),
                            )
                        if has_bias_abv:
                            nc.tensor.matmul(
                                dst, ones1[:], bcat[:], start=False, stop=True
                            )

                    # gd for pair i-2: keeps PE busy while softmax(i-1..i) runs
                    if i >= 2:
                        emit_gd(i - 2, first=(i == 2), last=False)

                    mw = mwts[i]
                    # exp(logit + SHIFT) -> fp16, straight into the M/W store
                    nc.scalar.activation(mw[:], p1v[:, :, 1:3, :], EXP,
                                         bias=shift[:])
                    sums = stp.tile([128, 2, 2], F32, tag="sums")
                    nc.vector.reduce_sum(sums[:], mw[:],
                                         axis=mybir.AxisListType.X)
                    rec = stp.tile([128, 2, 2], F32, tag="rec")
                    nc.vector.reciprocal(rec[:], sums[:])
                    # A scaled by 1/sum(expB) -- the entire B normalization
                    at = atp.tile([128, 2, 128], F16)
                    nc.vector.tensor_mul(
                        at[:], p1v[:, :, 0, :],
                        rec[:, :, 0:1].broadcast_to([128, 2, 128]),
                    )
                    ats[i] = at
                    # V half normalized in place (on the idle GPSIMD)
                    nc.gpsimd.tensor_mul(
                        mw[:, :, 1, :], mw[:, :, 1, :],
                        rec[:, :, 1:2].broadcast_to([128, 2, 128]),
                    )

                emit_gd(NP - 2, first=(NP == 2), last=False)
                emit_gd(NP - 1, first=False, last=True)
                gdts = gdsp.tile([128, 128], F16)
                nc.vector.tensor_copy(gdts[:], gdt[:])
                # G^T[k, c] = sum_m gd[m,k] wR[c,m] : one N=512 matmul
                gtp = pO.tile([128, 512], F32, tag="po", name=f"gtp{s}")
                nc.tensor.matmul(gtp[:], gdts[:], wrt[:], start=True, stop=True)
                gts = gtsp.tile([128, 512], F16)
                nc.scalar.copy(gts[:], gtp[:])

                # ---- phase 3: transpose W, then out = (wR gd) @ W ----
                wsbs = []
                osb = None

                def emit_group(g, osb_t):
                    half = (g % 2) * 512
                    for k in range(KC):
                        ock = pO.tile(
                            [128, 512], F32, tag="po", name=f"ock{s}_{g}_{k}"
                        )
                        nc.tensor.matmul(
                            ock[:], gts[:, k * 128:(k + 1) * 128], wsbs[g][:],
                            start=True, stop=True,
                        )
                        dst = osb_t[:, k, half:half + 512]
                        if has_bias_r:
                            nc.scalar.activation(
                                dst, ock[:], IDENT, bias=brt[:, k:k + 1]
                            )
                        elif k == 3 or (k == 2 and g % 2 == 1):
                            nc.vector.tensor_copy(dst, ock[:])
                        else:
                            nc.scalar.copy(dst, ock[:])
                    if g % 2 == 1:
                        dsto = out_d[s].rearrange("(k p) n -> p k n", p=128)
                        c0 = (g - 1) * 512
                        nc.sync.dma_start(dsto[:, :, c0:c0 + 1024], osb_t[:])

                for g in range(NG):
                    wpt = pW.tile([128, 512], F16)
                    for j in range(KC):
                        nt = 4 * g + j
                        nc.tensor.transpose(
                            wpt[:, j * 128:(j + 1) * 128],
                            mwts[nt // 2][:, nt % 2, 1, :],
                            ident16[:],
                        )
                    wsb = wsbp.tile([128, 512], F16, tag="wsb", name=f"wsb{s}_{g}")
                    nc.vector.tensor_copy(wsb[:], wpt[:])
                    wsbs.append(wsb)

                    if g % 2 == 0:
                        osb = osbp.tile(
                            [128, KC, 1024], F16, tag="osb", name=f"osb{s}_{g}"
                        )
                    if g > 0:
                        emit_group(g - 1, osb if g % 2 == 1 else prev_osb)
                    prev_osb = osb
                emit_group(NG - 1, osb)

    nc.compile()
    return nc


_CACHE = {}


def _get_nc(has_bias_abv: bool, has_bias_r: bool):
    key = (has_bias_abv, has_bias_r)
    if key not in _CACHE:
        _CACHE[key] = _build(*key)
    return _CACHE[key]


def _run(inputs, trace=False, **spmd_kwargs):
    x = np.asarray(inputs["x"])
    b, c, h, w = x.shape
    assert (b, c, h, w) == (B_GLOBAL, C_IN, H, W), x.shape
    wA = np.asarray(inputs["wA"], np.float32)
    wB = np.asarray(inputs["wB"], np.float32)
    wV = np.asarray(inputs["wV"], np.float32)
    wR = np.asarray(inputs["wR"], np.float32)
    bA = np.asarray(inputs["bA"], np.float32)
    bB = np.asarray(inputs["bB"], np.float32)
    bV = np.asarray(inputs["bV"], np.float32)
    bR = np.asarray(inputs["bR"], np.float32)

    has_bias_abv = bool(np.any(bA) or np.any(bB) or np.any(bV))
    has_bias_r = bool(np.any(bR))
    nc = _get_nc(has_bias_abv, has_bias_r)

    # [KC, 128, 384] : chunk k holds [wA.T | wB.T | wV.T][k*128:(k+1)*128, :]
    wcat = np.concatenate([wA.T, wB.T, wV.T], axis=1).reshape(KC, 128, 3 * 128)
    base = {
        "wcat": np.ascontiguousarray(wcat, dtype=np.float16),
        "wrt": np.ascontiguousarray(wR.T, dtype=np.float16),
    }
    if has_bias_abv:
        base["bcat"] = np.concatenate([bA, bB, bV])[None, :].astype(np.float16)
    if has_bias_r:
        base["brt"] = np.ascontiguousarray(bR.reshape(KC, 128).T, np.float32)

    xf = np.asarray(x, np.float16).reshape(B_GLOBAL, C_IN, N)
    in_maps = [
        dict(base, x=np.ascontiguousarray(xf[ci * B_LOC:(ci + 1) * B_LOC]))
        for ci in range(N_CORES)
    ]
    res = run_bass_kernel_spmd(
        nc, in_maps, core_ids=list(range(N_CORES)), trace=trace, **spmd_kwargs
    )
    out = np.concatenate(
        [res.results[ci]["out"].astype(np.float32) for ci in range(N_CORES)],
        axis=0,
    )
    return out.reshape(B_GLOBAL, C_IN, H, W), res


def kernel(**inputs):
    out, _ = _run(inputs)
    return out
